# revision 3
# baseline (speedup 1.0000x reference)
"""VQ codebook (nn_Codebook) Trainium2 kernel.

Full inputs: x [32, 256, 64, 64] f32, emb [1024, 256] f32.
Returns (x_q_out [32,256,64,64] f32, idx [131072] i32, loss f32 scalar)
matching jax reference:
    d = ||x||^2 + ||e||^2 - 2 x.e^T ; idx = argmin; x_q = emb[idx]
    loss = mean((x_q - x)^2) * (1 + 0.25) ; out = x + sg(x_q - x)  (== x_q)

Strategy (8 cores, data-parallel over batch, 4 batches/core):
  - scores: PSUM[-2m * 2^26] via PE (em2T = -2*embT*2^26 stationary-free rhs)
  - xsq * 2^26 per token via ACT Square + PE ones(2^26)-matmul + PE transpose
  - single-pass fused argmin on DVE custom op:
        key = ((esq' + xsq') - ((esq'+xsq') + s)) + Idx,  accum=MAX
    streamed reversed so Idx = 1023-k; key = (A-d)*2^26 + (1023-k) is an
    exact fp32 integer ((A-d) is a multiple of 2^-16 by fp32 lattice
    structure since d in [128,1024), |A-d| <= 0.25)
  - decode k* on DVE int ops; gather x_q columns from embT in SBUF via
    gpsimd ap_gather; loss reconstructed host-side from (maxkey, xsq, idx).
"""

import numpy as np

import concourse.bass as bass
import concourse.mybir as mybir
import concourse.tile as tile
from concourse import dve_ops, library_config
from concourse.dve_spec import Spec, Src0, Src1, C0, Idx, lower, maxx
from concourse.dve_uop import DveOpSpec
from concourse.bass_utils import run_bass_kernel_spmd

F32 = mybir.dt.float32
I32 = mybir.dt.int32
I16 = mybir.dt.int16
U16 = mybir.dt.uint16

B, C, H, W = 32, 256, 64, 64
K = 1024               # codebook size
NCORES = 8
BPC = B // NCORES      # batches per core
S = H * W              # spatial tokens per batch = 4096
NTILE = S // 128       # 32 token-tiles per batch
NGRP = NTILE // 4      # 8 groups of 4 tiles
BETA = 0.25
SC = np.float32(2.0 ** 26)


# ---------------------------------------------------------------- custom op
def _register_vqkey():
    name = "VQKEY2"
    for op in dve_ops.OPS:
        if op.name == name:
            return op
    _A = Src1 + C0
    _d = _A + Src0
    # r = xsq' - d  (per-token base, NOT A(k): A varies with k via esq and
    # would bias tie regions). Sterbenz-exact, multiple of 1024 in scaled units.
    spec = Spec(body=(C0 - _d) + Idx, accum=maxx)
    shas = {}
    for ver in ("v3", "v4"):
        try:
            uops = lower(spec, ver=ver)
            shas[ver] = DveOpSpec(
                name=name, opcode=0, uops=uops, rd1_en=dve_ops.has_src1(spec)
            ).sha(ver)
        except Exception:
            pass
    op = dve_ops.DveOp(name, spec, subdim=False, uops_sha=shas)
    dve_ops.OPS.append(op)
    dve_ops._SUB_OPCODE_FOR_NAME[name] = (
        dve_ops._CUSTOM_DVE_ROW_BASE + len(dve_ops.OPS) - 1
    )
    dve_ops.CUSTOM_DVE_SPECS[name] = spec
    return op


# ------------------------------------------------------- walrus workarounds
def _split_multiwaits(nc):
    """This walrus build embeds at most ONE sync-wait per instruction; move
    extra waits onto dedicated single-wait NoOps just before."""
    n = 0
    for fn in nc.m.functions:
        for blk in fn.blocks:
            new = []
            changed = False
            for inst in blk.instructions:
                si = inst.sync_info
                if si is not None and si.on_wait and len(si.on_wait) > 1:
                    waits = list(si.on_wait)
                    for w in waits[:-1]:
                        nop = mybir.InstNoOp(name=f"tw-{n}", ins=[], outs=[])
                        nop.engine = inst.engine
                        nop.sync_info = mybir.SyncInfo(on_wait=[w], on_update=[])
                        new.append(nop)
                        n += 1
                    si.on_wait = [waits[-1]]
                    inst.sync_info = si
                    changed = True
                new.append(inst)
            if changed:
                blk.instructions = new


# ------------------------------------------------------------- build kernel
def _build_nc():
    VQKEY = _register_vqkey()
    nc = bass.Bass()

    x_d = nc.dram_tensor("x", [BPC, 2, 128, S], F32, kind="ExternalInput")
    e2T_d = nc.dram_tensor("e2T", [2, 128, K], F32, kind="ExternalInput")
    eT_d = nc.dram_tensor("eT", [2, 128, K], F32, kind="ExternalInput")
    esqb_d = nc.dram_tensor("esqb", [128, K], F32, kind="ExternalInput")
    ones_d = nc.dram_tensor("ones26", [128, 1], F32, kind="ExternalInput")
    one1_d = nc.dram_tensor("one1", [1, 1], F32, kind="ExternalInput")

    out_d = nc.dram_tensor("out", [BPC, 2, 128, S], F32, kind="ExternalOutput")
    idx_d = nc.dram_tensor("idxo", [BPC, S], I32, kind="ExternalOutput")
    mk_d = nc.dram_tensor("mko", [BPC, S], F32, kind="ExternalOutput")
    xsq_d = nc.dram_tensor("xsqo", [BPC, S], F32, kind="ExternalOutput")

    with tile.TileContext(nc) as tc:
        with (
            tc.tile_pool(name="const", bufs=1) as constp,
            tc.tile_pool(name="xin", bufs=2) as xinp,
            tc.tile_pool(name="xqout", bufs=2) as xqp,
            tc.tile_pool(name="keyscr", bufs=1) as keyp,
            tc.tile_pool(name="small", bufs=2) as smallp,
            tc.tile_pool(name="ps_s", bufs=2, space="PSUM") as ps_s,
            tc.tile_pool(name="ps_row", bufs=1, space="PSUM") as ps_row,
            tc.tile_pool(name="ps_col", bufs=2, space="PSUM") as ps_col,
        ):
            nc.gpsimd.load_library(library_config.ap_gather)

            e2T_t = constp.tile([128, 2 * K], F32, name="e2T_t")
            eT_t = constp.tile([128, 2 * K], F32, name="eT_t")
            esqb_t = constp.tile([128, K], F32, name="esqb_t")
            ones_t = constp.tile([128, 1], F32, name="ones_t")
            one1_t = constp.tile([1, 1], F32, name="one1_t")
            nc.sync.dma_start(e2T_t[:, 0:K], e2T_d[0])
            nc.sync.dma_start(e2T_t[:, K:2 * K], e2T_d[1])
            nc.sync.dma_start(eT_t[:, 0:K], eT_d[0])
            nc.sync.dma_start(eT_t[:, K:2 * K], eT_d[1])
            nc.sync.dma_start(esqb_t[:], esqb_d[:])
            nc.sync.dma_start(ones_t[:], ones_d[:])
            nc.sync.dma_start(one1_t[:], one1_d[:])

            key_t = keyp.tile([128, K], F32, name="key_t")

            for b in range(BPC):
                xc = xinp.tile([128, 2 * S], F32, name="xc")
                nc.sync.dma_start(xc[:, 0:S], x_d[b, 0])
                nc.sync.dma_start(xc[:, S:2 * S], x_d[b, 1])

                xsqb = smallp.tile([128, NTILE], F32, name="xsqb")
                mkb = smallp.tile([128, NTILE], F32, name="mkb")

                for g in range(NGRP):
                    g0 = g * 512
                    # xsq' = sum_c x^2 * 2^26 for 512 tokens
                    sq0 = smallp.tile([128, 512], F32, name="sq0")
                    sq1 = smallp.tile([128, 512], F32, name="sq1")
                    nc.scalar.activation(
                        sq0[:], xc[:, g0:g0 + 512],
                        mybir.ActivationFunctionType.Square,
                    )
                    nc.scalar.activation(
                        sq1[:], xc[:, S + g0:S + g0 + 512],
                        mybir.ActivationFunctionType.Square,
                    )
                    xsqrow = ps_row.tile([1, 512], F32, name="xsqrow")
                    nc.tensor.matmul(xsqrow[:], ones_t[:], sq0[:],
                                     start=True, stop=False)
                    nc.tensor.matmul(xsqrow[:], ones_t[:], sq1[:],
                                     start=False, stop=True)
                    xsqrow_sb = smallp.tile([1, 512], F32, name="xsqrow_sb")
                    nc.scalar.activation(
                        xsqrow_sb[:], xsqrow[:],
                        mybir.ActivationFunctionType.Copy,
                    )
                    xsqcol = ps_col.tile([128, 4], F32, name="xsqcol")
                    for i in range(4):
                        nc.tensor.matmul(
                            xsqcol[:, i:i + 1],
                            xsqrow_sb[:, i * 128:(i + 1) * 128],
                            one1_t[:],
                            is_transpose=True,
                        )
                    nc.scalar.activation(
                        xsqb[:, g * 4:g * 4 + 4], xsqcol[:],
                        mybir.ActivationFunctionType.Copy,
                    )

                    for i in range(4):
                        t = g * 4 + i
                        t0 = t * 128
                        s_ps = ps_s.tile([128, K], F32, name="s_ps")
                        # -2m * 2^26 accumulated over the two C chunks
                        nc.tensor.matmul(s_ps[:, 0:512],
                                         xc[:, t0:t0 + 128],
                                         e2T_t[:, 0:512],
                                         start=True, stop=False)
                        nc.tensor.matmul(s_ps[:, 512:K],
                                         xc[:, t0:t0 + 128],
                                         e2T_t[:, 512:K],
                                         start=True, stop=False)
                        nc.tensor.matmul(s_ps[:, 0:512],
                                         xc[:, S + t0:S + t0 + 128],
                                         e2T_t[:, K:K + 512],
                                         start=False, stop=True)
                        nc.tensor.matmul(s_ps[:, 512:K],
                                         xc[:, S + t0:S + t0 + 128],
                                         e2T_t[:, K + 512:2 * K],
                                         start=False, stop=True)
                        nc.vector._custom_dve(
                            VQKEY,
                            out=key_t[:],
                            in0=s_ps[:, ::-1],
                            in1=esqb_t[:, ::-1],
                            s0=xsqb[:, t:t + 1],
                            accum_out=mkb[:, t:t + 1],
                        )

                # ---- decode k* = 1023 - (int(mk) & 1023)
                mki = smallp.tile([128, NTILE], I32, name="mki")
                nc.vector.tensor_copy(mki[:], mkb[:])
                ji = smallp.tile([128, NTILE], I32, name="ji")
                nc.vector.tensor_scalar(
                    ji[:], mki[:], 1023, None, op0=mybir.AluOpType.bitwise_and
                )
                jf = smallp.tile([128, NTILE], F32, name="jf")
                nc.vector.tensor_copy(jf[:], ji[:])
                kf = smallp.tile([128, NTILE], F32, name="kf")
                nc.vector.tensor_scalar(
                    kf[:], jf[:], -1.0, 1023.0,
                    op0=mybir.AluOpType.mult, op1=mybir.AluOpType.add,
                )
                ki = smallp.tile([128, NTILE], I32, name="ki")
                nc.vector.tensor_copy(ki[:], kf[:])
                k16 = smallp.tile([128, NTILE], I16, name="k16")
                nc.vector.tensor_copy(k16[:], kf[:])

                # ---- wrapped gather indices [128, 256] i16
                m16 = smallp.tile([16, 2 * NTILE * 4], I16, name="m16")
                idxs_t = smallp.tile([128, 2 * NTILE * 4], I16, name="idxs_t")
                for blk in range(8):
                    # M16[r, t*8+blk] = k16[blk*16+r, t]
                    dst = m16[:, blk::8]
                    nc.sync.dma_start(dst, k16[blk * 16:(blk + 1) * 16, :])
                for blk in range(8):
                    nc.sync.dma_start(idxs_t[blk * 16:(blk + 1) * 16, :], m16[:])

                # ---- gather x_q columns from embT
                xq = xqp.tile([128, 2 * S], F32, name="xq")
                nc.gpsimd.ap_gather(
                    xq[:, 0:S], eT_t[:, 0:K], idxs_t[:],
                    channels=128, num_elems=K, d=1, num_idxs=S,
                )
                nc.gpsimd.ap_gather(
                    xq[:, S:2 * S], eT_t[:, K:2 * K], idxs_t[:],
                    channels=128, num_elems=K, d=1, num_idxs=S,
                )
                nc.sync.dma_start(out_d[b, 0], xq[:, 0:S])
                nc.sync.dma_start(out_d[b, 1], xq[:, S:2 * S])

                # ---- small outputs: token order is s = t*128 + p
                idx_v = idx_d[b].rearrange("(t p) -> p t", p=128)
                nc.sync.dma_start(idx_v, ki[:])
                mk_v = mk_d[b].rearrange("(t p) -> p t", p=128)
                nc.sync.dma_start(mk_v, mkb[:])
                xsq_v = xsq_d[b].rearrange("(t p) -> p t", p=128)
                nc.sync.dma_start(xsq_v, xsqb[:])

    _split_multiwaits(nc)
    mybir.codegen_inst_isa_subclasses(nc)
    return nc


_NC_CACHE = None
TRACE = False
LAST_RESULT = None


def _get_nc():
    global _NC_CACHE
    if _NC_CACHE is None:
        _NC_CACHE = _build_nc()
    return _NC_CACHE


# ------------------------------------------------------------------- driver
def kernel(x: np.ndarray, emb: np.ndarray):
    x = np.ascontiguousarray(np.asarray(x, dtype=np.float32))
    emb = np.ascontiguousarray(np.asarray(emb, dtype=np.float32))
    assert x.shape == (B, C, H, W) and emb.shape == (K, C)

    embT = np.ascontiguousarray(emb.T)                      # [256, 1024]
    e2T = (embT.reshape(2, 128, K) * np.float32(-2.0) * SC).astype(np.float32)
    eT = np.ascontiguousarray(embT.reshape(2, 128, K))
    esq = (emb.astype(np.float32) ** 2).sum(axis=1, dtype=np.float32)
    esq_s = (esq * SC).astype(np.float32)
    esqb = np.tile(esq_s[None, :], (128, 1)).astype(np.float32)
    ones26 = np.full((128, 1), SC, dtype=np.float32)
    one1 = np.ones((1, 1), dtype=np.float32)

    xs = x.reshape(NCORES, BPC, 2, 128, S)

    in_maps = []
    for c in range(NCORES):
        in_maps.append({
            "x": np.ascontiguousarray(xs[c]),
            "e2T": e2T, "eT": eT, "esqb": esqb,
            "ones26": ones26, "one1": one1,
        })

    nc = _get_nc()
    res = run_bass_kernel_spmd(
        nc, in_maps, core_ids=list(range(NCORES)), trace=TRACE,
        trace_cores=list(range(NCORES)) if TRACE else None,
        stitch_traces=TRACE,
    )
    global LAST_RESULT
    LAST_RESULT = res

    outs = []
    idxs = []
    mks = []
    xsqs = []
    for c in range(NCORES):
        r = res.results[c]
        outs.append(r["out"].reshape(BPC, C, H, W))
        idxs.append(r["idxo"].reshape(-1))
        mks.append(r["mko"].reshape(-1))
        xsqs.append(r["xsqo"].reshape(-1))
    x_q_out = np.concatenate(outs, axis=0).astype(np.float32)
    idx = np.concatenate(idxs).astype(np.int32)
    mk = np.concatenate(mks)
    xsq_s = np.concatenate(xsqs)

    # ---- loss from packed keys: key = Kq*1024 + j, d*2^26 = xsq' - Kq*2^10
    mki = mk.astype(np.int64)
    j = mki & 1023
    Kq = (mki - j) >> 10
    d_scaled = xsq_s.astype(np.float64) - (Kq.astype(np.float64) * 1024.0)
    d = d_scaled * (2.0 ** -26)
    m1 = np.float32(d.sum() / (B * H * W * C))
    loss = np.float32(m1 + np.float32(np.float32(BETA) * m1))
    return x_q_out, idx, loss
